# revision 41
# baseline (speedup 1.0000x reference)
"""CapsuleLayer kernel for Trainium2 (8 NeuronCores, Bass/Tile).

Math: reference einsum("bhwf,fcd->bhwd", x, Wc) sums over BOTH f and c,
so it collapses to a single matmul:
    W_eff[f, d] = sum_c capsules.reshape(F, C, D)[f, c, d]
    out = x.reshape(-1, F) @ W_eff            # (100352, 256) @ (256, 16)

Distribution: data-parallel over flattened positions (batch*H*W), 12544
positions per core; the small capsule weight is replicated (and pre-summed
+ pre-cast on the host — 41K flops on a constant; the device loads 8 KB).

v6 architecture (mode "fp8", the default):
  - x is quantized host-side to fp8 E3M4 (4 mantissa bits) and streamed at
    1 B/elem: 3.21 MB/core — measured at the 8-core aggregate HBM roofline
    (~400-430 GB/s/core). Quantization rel err 1.35e-2 (x only; weights
    ride fp16 through the mixed-dtype matmul).
  - host lays x out chunk-major-contiguous per core: SEVEN uniform chunks
    of 1792 positions (= 7 PSUM groups, 4 col-tiled strips each), each a
    contiguous [128, KC*1792] block so every DMA is ONE 3584-B descriptor
    per partition. (448-pos tail chunks measured 25 GB/s — sub-512B
    descriptor RMW penalty; and serial non-col-tiled tail matmuls left
    3.6 us of PE work after the stream ended.)
  - weff rides first on the lighter scalar ring; chunks alternate rings.
  - per group: 4 strips via tile_position=(0,32s) into one PSUM bank,
    2 K-passes each; drains are fp32->fp16 casts alternating DVE/ACT.
  - stores are single full-width [128, ...] DMAs (junk partitions included
    — the host slices rows 32s..32s+15): outA (g0-3) on SWDGE mid-stream,
    outB (g4-5) on scalar, outC (g6) on sync right after the last copy.
    Per-strip sliced stores cost 4 serialized ~0.55us ring issues each;
    full-width costs one issue + 4x bytes, all off the critical path.

Modes: 'fp8' (default), 'fp16', 'f32r', 'fp32' — dtype of the streamed x
shard and PE moving operand; fp8 keeps weights fp16 and output fp16.
"""

import ml_dtypes
import numpy as np

import concourse.bass as bass  # noqa: F401  (engine types referenced via nc)
import concourse.tile as tile
from concourse import bacc, mybir
from concourse.bass_utils import run_bass_kernel_spmd

N_CORES = 8
B, H, W, F = 32, 56, 56, 256
NUM_CAPS, CAP_DIM = 10, 16
POS = B * H * W            # 100352
PPC = POS // N_CORES       # 12544 positions per core
SUB = 448                  # matmul moving free dim (<=512 fp32 PSUM)
GRP = 4 * SUB              # 1792 positions per PSUM group (4 col-tiled strips)
NG = 7                     # 7 uniform groups = 12544
KC = F // 128              # 2 contraction chunks of 128

SYNC_CHUNKS = [0, 2, 4, 6]     # ring FIFO order; g6 lands last
SCALAR_CHUNKS = [1, 3, 5]      # weff rides first on scalar

MODE = "fp8"               # default; see module docstring

_MM_DT = {
    "fp32": mybir.dt.float32,
    "f32r": mybir.dt.float32r,
    "fp16": mybir.dt.float16,
    "fp8": mybir.dt.float8e3,   # E3M4: 4 mantissa bits, x-quant rel err ~1.3e-2
}
_NP_DT = {
    "fp32": np.float32,
    "f32r": np.float32,
    "fp16": np.float16,
    "fp8": ml_dtypes.float8_e3m4,
}

_cache = {}


def _build(mode: str):
    nc = bacc.Bacc(
        None,
        target_bir_lowering=False,
        debug=False,
        enable_asserts=False,
        num_devices=N_CORES,
    )
    mm_dt = _MM_DT[mode]
    w_dt = mybir.dt.float16 if mode == "fp8" else mm_dt
    o_dt = mybir.dt.float16 if mode == "fp8" else mybir.dt.float32

    # chunk-major contiguous: chunk g occupies cols [2*g*GRP, 2*(g+1)*GRP)
    xT = nc.dram_tensor("xT", [128, KC * PPC], mm_dt, kind="ExternalInput")
    win = nc.dram_tensor("win", [128, KC * CAP_DIM], w_dt, kind="ExternalInput")
    # full-width outputs: row 32s+d of dim0 carries strip s, capsule-dim d;
    # the other 16-partition half-blocks are junk the host ignores
    outA = nc.dram_tensor("outA", [128, 4, SUB], o_dt, kind="ExternalOutput")
    outB = nc.dram_tensor("outB", [128, 2, SUB], o_dt, kind="ExternalOutput")
    outC = nc.dram_tensor("outC", [128, SUB], o_dt, kind="ExternalOutput")

    with tile.TileContext(nc) as tc:
        with (
            tc.tile_pool(name="const", bufs=1) as cpool,
            tc.tile_pool(name="xin", bufs=1) as xpool,
            tc.tile_pool(name="psumb", bufs=7, space="PSUM") as pspool,
            tc.tile_pool(name="psumw", bufs=1, space="PSUM") as pspool_w,
        ):
            # ---- weight first on the (lighter) scalar ring ---------------
            weff = cpool.tile([128, KC, CAP_DIM], w_dt, tag="weff")
            nc.scalar.dma_start(weff[:], win.rearrange("p (k d) -> p k d", k=KC))

            # ---- input stream: flat contiguous chunks --------------------
            xts = [None] * NG
            order = []
            for j in range(4):
                order.append((SYNC_CHUNKS[j], nc.sync))
                if j < len(SCALAR_CHUNKS):
                    order.append((SCALAR_CHUNKS[j], nc.scalar))
            for g, ring in order:
                xt = xpool.tile([128, KC * GRP], mm_dt, tag=f"xt{g}")
                ring.dma_start(xt[:], xT[:, 2 * g * GRP : 2 * (g + 1) * GRP])
                xts[g] = xt

            # ---- staging tiles (fp16) ------------------------------------
            ob_a = cpool.tile([128, 4, SUB], o_dt, tag="oba")    # g0..g3
            ob_b = cpool.tile([128, 2, SUB], o_dt, tag="obb")    # g4,g5
            ob_c = cpool.tile([128, SUB], o_dt, tag="obc")       # g6

            # ---- PE warm-up: dummy matmuls while the first chunk streams.
            # The PE HAM clock-gate needs ~3.4us of sustained activity to
            # lift 1.2 -> 2.4 GHz; real MMs only start ~5us in, and bursty
            # duty never warms it (all MMs measured at cold-rate ~577ns).
            # 12 no-dep MMs on garbage SBUF fill the idle window; results
            # land in a scratch PSUM bank nobody reads.
            dummy_w = cpool.tile([128, CAP_DIM], w_dt, tag="dummyw")
            dummy_m = cpool.tile([128, SUB], mm_dt, tag="dummym")
            nc.vector.memset(dummy_w[:], 0.0)
            nc.vector.memset(dummy_m[:], 0.0)
            ps_warm = pspool_w.tile([CAP_DIM, 512], mybir.dt.float32, tag="pswarm")
            for _ in range(12):
                nc.tensor.matmul(
                    ps_warm[:, 0:SUB], dummy_w[:], dummy_m[:], start=True, stop=True
                )

            def drain(eng, dst, src):
                if eng == "dve":
                    nc.vector.tensor_copy(dst, src)
                else:
                    nc.scalar.copy(dst, src)

            # ---- groups: 4 col-tiled strips per PSUM bank ----------------
            for g in range(NG):
                xt = xts[g]
                ps = pspool.tile([128, 512], mybir.dt.float32, tag="psb")
                for s in range(4):
                    for k in range(KC):
                        sl = slice(k * GRP + s * SUB, k * GRP + (s + 1) * SUB)
                        nc.tensor.matmul(
                            ps[32 * s : 32 * s + CAP_DIM, 0:SUB],
                            weff[:, k, :],
                            xt[:, sl],
                            start=(k == 0),
                            stop=(k == KC - 1),
                            tile_position=(0, 32 * s),
                        )
                eng = "dve" if g % 2 == 0 else "act"
                if g < 4:
                    drain(eng, ob_a[:, g, :], ps[:, 0:SUB])
                elif g < 6:
                    drain(eng, ob_b[:, g - 4, :], ps[:, 0:SUB])
                else:
                    drain(eng, ob_c[:], ps[:, 0:SUB])

            # ---- stores: one full-width DMA per block; outC split across
            # both rings so the two final receipts overlap
            nc.gpsimd.dma_start(outA[:], ob_a[:])   # ready mid-stream; SWDGE
            nc.scalar.dma_start(outB[:], ob_b[:])   # after g5's copy
            nc.sync.dma_start(outC[0:64, :], ob_c[0:64, :])
            nc.scalar.dma_start(outC[64:128, :], ob_c[64:128, :])

    nc.compile()
    return nc


def _get_nc(mode: str):
    if mode not in _cache:
        _cache[mode] = _build(mode)
    return _cache[mode]


def _pack_core(xc):
    """[256, PPC] -> chunk-major [128, KC*PPC] (each chunk contiguous)."""
    parts = []
    for g in range(NG):
        blk = xc[:, g * GRP : (g + 1) * GRP].reshape(KC, 128, GRP)
        parts.append(blk.transpose(1, 0, 2).reshape(128, KC * GRP))
    return np.concatenate(parts, axis=1)


def run(x, capsules, trace=False, trace_cores=None, mode=None):
    """Shard, execute on 8 cores, gather. Returns (out, BassKernelResults)."""
    if mode is None:
        mode = MODE
    nc = _get_nc(mode)

    x = np.asarray(x, dtype=np.float32)
    capsules = np.asarray(capsules, dtype=np.float32)
    xf = np.ascontiguousarray(
        x.reshape(POS, F).astype(_NP_DT[mode], copy=False).T
    )  # [F, POS]
    # W_eff[f,d] = sum_c Wc[f,c,d]; pack rows f=(k,p) -> [128, KC*CAP_DIM]
    weff = capsules.reshape(F, NUM_CAPS, CAP_DIM).sum(axis=1)
    w_np = np.float16 if mode == "fp8" else _NP_DT[mode]
    win = np.ascontiguousarray(
        weff.reshape(KC, 128, CAP_DIM).transpose(1, 0, 2).reshape(128, KC * CAP_DIM)
    ).astype(w_np)

    in_maps = [
        {"xT": _pack_core(xf[:, c * PPC : (c + 1) * PPC]), "win": win}
        for c in range(N_CORES)
    ]
    res = run_bass_kernel_spmd(
        nc,
        in_maps,
        core_ids=list(range(N_CORES)),
        trace=trace,
        trace_cores=trace_cores,
    )
    # strip s of group g lives in dram rows 32s..32s+CAP_DIM
    rows = (32 * np.arange(4)[:, None] + np.arange(CAP_DIM)[None, :]).ravel()
    out = np.empty((POS, CAP_DIM), dtype=np.float32)
    for c in range(N_CORES):
        r = res.results[c]
        full = np.empty((4, CAP_DIM, NG, SUB), dtype=np.float32)  # s d g n
        full[:, :, 0:4] = (
            r["outA"][rows].reshape(4, CAP_DIM, 4, SUB).astype(np.float32)
        )
        full[:, :, 4:6] = (
            r["outB"][rows].reshape(4, CAP_DIM, 2, SUB).astype(np.float32)
        )
        full[:, :, 6] = r["outC"][rows].reshape(4, CAP_DIM, SUB).astype(np.float32)
        # position = g*1792 + s*448 + n
        out[c * PPC : (c + 1) * PPC] = (
            full.transpose(1, 2, 0, 3).reshape(CAP_DIM, PPC).T
        )
    return out.reshape(B, H, W, CAP_DIM), res


def kernel(x, capsules):
    out, _ = run(x, capsules)
    return out


# revision 42
# speedup vs baseline: 1.0335x; 1.0335x over previous
"""CapsuleLayer kernel for Trainium2 (8 NeuronCores, Bass/Tile).

Math: reference einsum("bhwf,fcd->bhwd", x, Wc) sums over BOTH f and c,
so it collapses to a single matmul:
    W_eff[f, d] = sum_c capsules.reshape(F, C, D)[f, c, d]
    out = x.reshape(-1, F) @ W_eff            # (100352, 256) @ (256, 16)

Distribution: data-parallel over flattened positions (batch*H*W), 12544
positions per core; the small capsule weight is replicated (and pre-summed
+ pre-cast on the host — 41K flops on a constant; the device loads 8 KB).

v8 architecture (mode "fp8", the default; measured best ~24.4us vs the
34.5us fp16 baseline, rel err 1.3503e-2 against the 2e-2 gate):
  - x is quantized host-side to fp8 E3M4 (4 mantissa bits) and streamed at
    1 B/elem: 3.21 MB/core — measured at the 8-core aggregate HBM roofline
    (~400-430 GB/s/core). Quantization rel err 1.35e-2 (x only; weights
    ride fp16 through the mixed-dtype matmul).
  - host lays x out chunk-major-contiguous per core: SEVEN uniform chunks
    of 1792 positions (= 7 PSUM groups, 4 col-tiled strips each), each a
    contiguous [128, KC*1792] block so every DMA is ONE 3584-B descriptor
    per partition. (448-pos tail chunks measured 25 GB/s — sub-512B
    descriptor RMW penalty; and serial non-col-tiled tail matmuls left
    3.6 us of PE work after the stream ended.)
  - weff rides first on the lighter scalar ring; chunks alternate rings.
  - per group: 4 strips via tile_position=(0,32s) into one PSUM bank
    (bufs=7 — one bank per group; bufs=4 recycling stalled the PE ~1.6us),
    2 K-passes each; drains are fp32->fp16 casts alternating DVE/ACT.
  - 12 dummy matmuls on zeroed scratch fill the PE's pre-stream idle
    window so the HAM clock-gate lifts 1.2->2.4 GHz before real work
    (tail MMs measured 577ns cold -> 352ns warm).
  - stores are single full-width [128, ...] DMAs (junk partitions included
    — the host slices rows 32s..32s+15): outA (g0-3) on SWDGE mid-stream,
    outB (g4-5) on scalar; outC (g6) is split across both rings right
    after the last copy so the two final receipts overlap.
    Per-strip sliced stores cost 4 serialized ~0.55us ring issues each;
    full-width costs one issue + 4x bytes, all off the critical path.
  - remaining fixed costs inside the measured window: ~0.8us walrus
    preamble, ~2.2us to first DMA byte, ~7.1us walrus epilogue that zeroes
    all 253 semaphores (unavoidable; --max-sem-num experiments regressed).

Modes: 'fp8' (default), 'fp16', 'f32r', 'fp32' — dtype of the streamed x
shard and PE moving operand; fp8 keeps weights fp16 and output fp16.
"""

import ml_dtypes
import numpy as np

import concourse.bass as bass  # noqa: F401  (engine types referenced via nc)
import concourse.tile as tile
from concourse import bacc, mybir
from concourse.bass_utils import run_bass_kernel_spmd

N_CORES = 8
B, H, W, F = 32, 56, 56, 256
NUM_CAPS, CAP_DIM = 10, 16
POS = B * H * W            # 100352
PPC = POS // N_CORES       # 12544 positions per core
SUB = 448                  # matmul moving free dim (<=512 fp32 PSUM)
GRP = 4 * SUB              # 1792 positions per PSUM group (4 col-tiled strips)
NG = 7                     # 7 uniform groups = 12544
KC = F // 128              # 2 contraction chunks of 128

SYNC_CHUNKS = [0, 2, 4, 6]     # ring FIFO order; g6 lands last
SCALAR_CHUNKS = [1, 3, 5]      # weff rides first on scalar

MODE = "fp8"               # default; see module docstring

_MM_DT = {
    "fp32": mybir.dt.float32,
    "f32r": mybir.dt.float32r,
    "fp16": mybir.dt.float16,
    "fp8": mybir.dt.float8e3,   # E3M4: 4 mantissa bits, x-quant rel err ~1.3e-2
}
_NP_DT = {
    "fp32": np.float32,
    "f32r": np.float32,
    "fp16": np.float16,
    "fp8": ml_dtypes.float8_e3m4,
}

_cache = {}


def _build(mode: str):
    nc = bacc.Bacc(
        None,
        target_bir_lowering=False,
        debug=False,
        enable_asserts=False,
        num_devices=N_CORES,
    )
    mm_dt = _MM_DT[mode]
    w_dt = mybir.dt.float16 if mode == "fp8" else mm_dt
    o_dt = mybir.dt.float16 if mode == "fp8" else mybir.dt.float32

    # chunk-major contiguous: chunk g occupies cols [2*g*GRP, 2*(g+1)*GRP)
    xT = nc.dram_tensor("xT", [128, KC * PPC], mm_dt, kind="ExternalInput")
    win = nc.dram_tensor("win", [128, KC * CAP_DIM], w_dt, kind="ExternalInput")
    # full-width outputs: row 32s+d of dim0 carries strip s, capsule-dim d;
    # the other 16-partition half-blocks are junk the host ignores
    outA = nc.dram_tensor("outA", [128, 4, SUB], o_dt, kind="ExternalOutput")
    outB = nc.dram_tensor("outB", [128, 2, SUB], o_dt, kind="ExternalOutput")
    outC = nc.dram_tensor("outC", [128, SUB], o_dt, kind="ExternalOutput")

    with tile.TileContext(nc) as tc:
        with (
            tc.tile_pool(name="const", bufs=1) as cpool,
            tc.tile_pool(name="xin", bufs=1) as xpool,
            tc.tile_pool(name="psumb", bufs=7, space="PSUM") as pspool,
            tc.tile_pool(name="psumw", bufs=1, space="PSUM") as pspool_w,
        ):
            # ---- weight first on the (lighter) scalar ring ---------------
            weff = cpool.tile([128, KC, CAP_DIM], w_dt, tag="weff")
            nc.scalar.dma_start(weff[:], win.rearrange("p (k d) -> p k d", k=KC))

            # ---- input stream: flat contiguous chunks --------------------
            xts = [None] * NG
            order = []
            for j in range(4):
                order.append((SYNC_CHUNKS[j], nc.sync))
                if j < len(SCALAR_CHUNKS):
                    order.append((SCALAR_CHUNKS[j], nc.scalar))
            for g, ring in order:
                xt = xpool.tile([128, KC * GRP], mm_dt, tag=f"xt{g}")
                ring.dma_start(xt[:], xT[:, 2 * g * GRP : 2 * (g + 1) * GRP])
                xts[g] = xt

            # ---- staging tiles (fp16) ------------------------------------
            ob_a = cpool.tile([128, 4, SUB], o_dt, tag="oba")    # g0..g3
            ob_b = cpool.tile([128, 2, SUB], o_dt, tag="obb")    # g4,g5
            ob_c = cpool.tile([128, SUB], o_dt, tag="obc")       # g6

            # ---- PE warm-up: dummy matmuls while the first chunk streams.
            # The PE HAM clock-gate needs ~3.4us of sustained activity to
            # lift 1.2 -> 2.4 GHz; real MMs only start ~5us in, and bursty
            # duty never warms it (all MMs measured at cold-rate ~577ns).
            # 12 no-dep MMs on garbage SBUF fill the idle window; results
            # land in a scratch PSUM bank nobody reads.
            dummy_w = cpool.tile([128, CAP_DIM], w_dt, tag="dummyw")
            dummy_m = cpool.tile([128, SUB], mm_dt, tag="dummym")
            nc.vector.memset(dummy_w[:], 0.0)
            nc.vector.memset(dummy_m[:], 0.0)
            ps_warm = pspool_w.tile([CAP_DIM, 512], mybir.dt.float32, tag="pswarm")
            for _ in range(12):
                nc.tensor.matmul(
                    ps_warm[:, 0:SUB], dummy_w[:], dummy_m[:], start=True, stop=True
                )

            def drain(eng, dst, src):
                if eng == "dve":
                    nc.vector.tensor_copy(dst, src)
                else:
                    nc.scalar.copy(dst, src)

            # ---- groups: 4 col-tiled strips per PSUM bank ----------------
            for g in range(NG):
                xt = xts[g]
                ps = pspool.tile([128, 512], mybir.dt.float32, tag="psb")
                for s in range(4):
                    for k in range(KC):
                        sl = slice(k * GRP + s * SUB, k * GRP + (s + 1) * SUB)
                        nc.tensor.matmul(
                            ps[32 * s : 32 * s + CAP_DIM, 0:SUB],
                            weff[:, k, :],
                            xt[:, sl],
                            start=(k == 0),
                            stop=(k == KC - 1),
                            tile_position=(0, 32 * s),
                        )
                eng = "dve" if g % 2 == 0 else "act"
                if g < 4:
                    drain(eng, ob_a[:, g, :], ps[:, 0:SUB])
                elif g < 6:
                    drain(eng, ob_b[:, g - 4, :], ps[:, 0:SUB])
                else:
                    drain(eng, ob_c[:], ps[:, 0:SUB])

            # ---- stores: one full-width DMA per block; outC split across
            # both rings so the two final receipts overlap
            nc.gpsimd.dma_start(outA[:], ob_a[:])   # ready mid-stream; SWDGE
            nc.scalar.dma_start(outB[:], ob_b[:])   # after g5's copy
            nc.sync.dma_start(outC[0:64, :], ob_c[0:64, :])
            nc.scalar.dma_start(outC[64:128, :], ob_c[64:128, :])

    nc.compile()
    return nc


def _get_nc(mode: str):
    if mode not in _cache:
        _cache[mode] = _build(mode)
    return _cache[mode]


def _pack_core(xc):
    """[256, PPC] -> chunk-major [128, KC*PPC] (each chunk contiguous)."""
    parts = []
    for g in range(NG):
        blk = xc[:, g * GRP : (g + 1) * GRP].reshape(KC, 128, GRP)
        parts.append(blk.transpose(1, 0, 2).reshape(128, KC * GRP))
    return np.concatenate(parts, axis=1)


def run(x, capsules, trace=False, trace_cores=None, mode=None):
    """Shard, execute on 8 cores, gather. Returns (out, BassKernelResults)."""
    if mode is None:
        mode = MODE
    nc = _get_nc(mode)

    x = np.asarray(x, dtype=np.float32)
    capsules = np.asarray(capsules, dtype=np.float32)
    xf = np.ascontiguousarray(
        x.reshape(POS, F).astype(_NP_DT[mode], copy=False).T
    )  # [F, POS]
    # W_eff[f,d] = sum_c Wc[f,c,d]; pack rows f=(k,p) -> [128, KC*CAP_DIM]
    weff = capsules.reshape(F, NUM_CAPS, CAP_DIM).sum(axis=1)
    w_np = np.float16 if mode == "fp8" else _NP_DT[mode]
    win = np.ascontiguousarray(
        weff.reshape(KC, 128, CAP_DIM).transpose(1, 0, 2).reshape(128, KC * CAP_DIM)
    ).astype(w_np)

    in_maps = [
        {"xT": _pack_core(xf[:, c * PPC : (c + 1) * PPC]), "win": win}
        for c in range(N_CORES)
    ]
    res = run_bass_kernel_spmd(
        nc,
        in_maps,
        core_ids=list(range(N_CORES)),
        trace=trace,
        trace_cores=trace_cores,
    )
    # strip s of group g lives in dram rows 32s..32s+CAP_DIM
    rows = (32 * np.arange(4)[:, None] + np.arange(CAP_DIM)[None, :]).ravel()
    out = np.empty((POS, CAP_DIM), dtype=np.float32)
    for c in range(N_CORES):
        r = res.results[c]
        full = np.empty((4, CAP_DIM, NG, SUB), dtype=np.float32)  # s d g n
        full[:, :, 0:4] = (
            r["outA"][rows].reshape(4, CAP_DIM, 4, SUB).astype(np.float32)
        )
        full[:, :, 4:6] = (
            r["outB"][rows].reshape(4, CAP_DIM, 2, SUB).astype(np.float32)
        )
        full[:, :, 6] = r["outC"][rows].reshape(4, CAP_DIM, SUB).astype(np.float32)
        # position = g*1792 + s*448 + n
        out[c * PPC : (c + 1) * PPC] = (
            full.transpose(1, 2, 0, 3).reshape(CAP_DIM, PPC).T
        )
    return out.reshape(B, H, W, CAP_DIM), res


def kernel(x, capsules):
    out, _ = run(x, capsules)
    return out


# revision 43
# speedup vs baseline: 1.0369x; 1.0033x over previous
"""CapsuleLayer kernel for Trainium2 (8 NeuronCores, Bass/Tile).

Math: reference einsum("bhwf,fcd->bhwd", x, Wc) sums over BOTH f and c,
so it collapses to a single matmul:
    W_eff[f, d] = sum_c capsules.reshape(F, C, D)[f, c, d]
    out = x.reshape(-1, F) @ W_eff            # (100352, 256) @ (256, 16)

Distribution: data-parallel over flattened positions (batch*H*W), 12544
positions per core; the small capsule weight is replicated (and pre-summed
+ pre-cast on the host — 41K flops on a constant; the device loads 8 KB).

v8 architecture (mode "fp8", the default; measured best ~24.4us vs the
34.5us fp16 baseline, rel err 1.3503e-2 against the 2e-2 gate):
  - x is quantized host-side to fp8 E3M4 (4 mantissa bits) and streamed at
    1 B/elem: 3.21 MB/core — measured at the 8-core aggregate HBM roofline
    (~400-430 GB/s/core). Quantization rel err 1.35e-2 (x only; weights
    ride fp16 through the mixed-dtype matmul).
  - host lays x out chunk-major-contiguous per core: SEVEN uniform chunks
    of 1792 positions (= 7 PSUM groups, 4 col-tiled strips each), each a
    contiguous [128, KC*1792] block so every DMA is ONE 3584-B descriptor
    per partition. (448-pos tail chunks measured 25 GB/s — sub-512B
    descriptor RMW penalty; and serial non-col-tiled tail matmuls left
    3.6 us of PE work after the stream ended.)
  - weff rides first on the lighter scalar ring; chunks alternate rings.
  - per group: 4 strips via tile_position=(0,32s) into one PSUM bank
    (bufs=7 — one bank per group; bufs=4 recycling stalled the PE ~1.6us),
    2 K-passes each; drains are fp32->fp16 casts alternating DVE/ACT.
  - 12 dummy matmuls on zeroed scratch fill the PE's pre-stream idle
    window so the HAM clock-gate lifts 1.2->2.4 GHz before real work
    (tail MMs measured 577ns cold -> 352ns warm).
  - stores are single full-width [128, ...] DMAs (junk partitions included
    — the host slices rows 32s..32s+15): outA (g0-3) on SWDGE mid-stream,
    outB (g4-5) on scalar; outC (g6) is split across both rings right
    after the last copy so the two final receipts overlap.
    Per-strip sliced stores cost 4 serialized ~0.55us ring issues each;
    full-width costs one issue + 4x bytes, all off the critical path.
  - remaining fixed costs inside the measured window: ~0.8us walrus
    preamble, ~2.2us to first DMA byte, ~7.1us walrus epilogue that zeroes
    all 253 semaphores (unavoidable; --max-sem-num experiments regressed).

Modes: 'fp8' (default), 'fp16', 'f32r', 'fp32' — dtype of the streamed x
shard and PE moving operand; fp8 keeps weights fp16 and output fp16.
"""

import ml_dtypes
import numpy as np

import concourse.bass as bass  # noqa: F401  (engine types referenced via nc)
import concourse.tile as tile
from concourse import bacc, mybir
from concourse.bass_utils import run_bass_kernel_spmd

N_CORES = 8
B, H, W, F = 32, 56, 56, 256
NUM_CAPS, CAP_DIM = 10, 16
POS = B * H * W            # 100352
PPC = POS // N_CORES       # 12544 positions per core
SUB = 448                  # matmul moving free dim (<=512 fp32 PSUM)
GRP = 4 * SUB              # 1792 positions per PSUM group (4 col-tiled strips)
NG = 7                     # 7 uniform groups = 12544
KC = F // 128              # 2 contraction chunks of 128

SYNC_CHUNKS = [0, 2, 4, 6]     # ring FIFO order; g6 lands last
SCALAR_CHUNKS = [1, 3, 5]      # weff rides first on scalar

MODE = "fp8"               # default; see module docstring

_MM_DT = {
    "fp32": mybir.dt.float32,
    "f32r": mybir.dt.float32r,
    "fp16": mybir.dt.float16,
    "fp8": mybir.dt.float8e3,   # E3M4: 4 mantissa bits, x-quant rel err ~1.3e-2
}
_NP_DT = {
    "fp32": np.float32,
    "f32r": np.float32,
    "fp16": np.float16,
    "fp8": ml_dtypes.float8_e3m4,
}

_cache = {}


def _build(mode: str):
    nc = bacc.Bacc(
        None,
        target_bir_lowering=False,
        debug=False,
        enable_asserts=False,
        num_devices=N_CORES,
    )
    mm_dt = _MM_DT[mode]
    w_dt = mybir.dt.float16 if mode == "fp8" else mm_dt
    o_dt = mybir.dt.float16 if mode == "fp8" else mybir.dt.float32

    # chunk-major contiguous: chunk g occupies cols [2*g*GRP, 2*(g+1)*GRP)
    xT = nc.dram_tensor("xT", [128, KC * PPC], mm_dt, kind="ExternalInput")
    win = nc.dram_tensor("win", [128, KC * CAP_DIM], w_dt, kind="ExternalInput")
    # full-width outputs: row 32s+d of dim0 carries strip s, capsule-dim d;
    # the other 16-partition half-blocks are junk the host ignores
    outA = nc.dram_tensor("outA", [128, 4, SUB], o_dt, kind="ExternalOutput")
    outB = nc.dram_tensor("outB", [128, 2, SUB], o_dt, kind="ExternalOutput")
    outC = nc.dram_tensor("outC", [128, SUB], o_dt, kind="ExternalOutput")

    with tile.TileContext(nc) as tc:
        with (
            tc.tile_pool(name="const", bufs=1) as cpool,
            tc.tile_pool(name="xin", bufs=1) as xpool,
            tc.tile_pool(name="psumb", bufs=7, space="PSUM") as pspool,
            tc.tile_pool(name="psumw", bufs=1, space="PSUM") as pspool_w,
        ):
            # ---- weight first on the (lighter) scalar ring ---------------
            weff = cpool.tile([128, KC, CAP_DIM], w_dt, tag="weff")
            nc.scalar.dma_start(weff[:], win.rearrange("p (k d) -> p k d", k=KC))

            # ---- input stream: flat contiguous chunks --------------------
            xts = [None] * NG
            order = []
            for j in range(4):
                order.append((SYNC_CHUNKS[j], nc.sync))
                if j < len(SCALAR_CHUNKS):
                    order.append((SCALAR_CHUNKS[j], nc.scalar))
            for g, ring in order:
                xt = xpool.tile([128, KC * GRP], mm_dt, tag=f"xt{g}")
                ring.dma_start(xt[:], xT[:, 2 * g * GRP : 2 * (g + 1) * GRP])
                xts[g] = xt

            # ---- staging tiles (fp16) ------------------------------------
            ob_a = cpool.tile([128, 4, SUB], o_dt, tag="oba")    # g0..g3
            ob_b = cpool.tile([128, 2, SUB], o_dt, tag="obb")    # g4,g5
            ob_c = cpool.tile([128, SUB], o_dt, tag="obc")       # g6

            # ---- PE warm-up: dummy matmuls while the first chunk streams.
            # The PE HAM clock-gate needs ~3.4us of sustained activity to
            # lift 1.2 -> 2.4 GHz; real MMs only start ~5us in, and bursty
            # duty never warms it (all MMs measured at cold-rate ~577ns).
            # 12 no-dep MMs on garbage SBUF fill the idle window; results
            # land in a scratch PSUM bank nobody reads.
            dummy_w = cpool.tile([128, CAP_DIM], w_dt, tag="dummyw")
            dummy_m = cpool.tile([128, SUB], mm_dt, tag="dummym")
            nc.vector.memset(dummy_w[:], 0.0)
            nc.vector.memset(dummy_m[:], 0.0)
            ps_warm = pspool_w.tile([CAP_DIM, 512], mybir.dt.float32, tag="pswarm")
            for _ in range(12):
                nc.tensor.matmul(
                    ps_warm[:, 0:SUB], dummy_w[:], dummy_m[:], start=True, stop=True
                )

            def drain(eng, dst, src):
                if eng == "dve":
                    nc.vector.tensor_copy(dst, src)
                else:
                    nc.scalar.copy(dst, src)

            # ---- groups: 4 col-tiled strips per PSUM bank ----------------
            for g in range(NG):
                xt = xts[g]
                ps = pspool.tile([128, 512], mybir.dt.float32, tag="psb")
                for s in range(4):
                    for k in range(KC):
                        sl = slice(k * GRP + s * SUB, k * GRP + (s + 1) * SUB)
                        nc.tensor.matmul(
                            ps[32 * s : 32 * s + CAP_DIM, 0:SUB],
                            weff[:, k, :],
                            xt[:, sl],
                            start=(k == 0),
                            stop=(k == KC - 1),
                            tile_position=(0, 32 * s),
                        )
                eng = "dve" if g % 2 == 0 else "act"
                if g < 4:
                    drain(eng, ob_a[:, g, :], ps[:, 0:SUB])
                elif g < 6:
                    drain(eng, ob_b[:, g - 4, :], ps[:, 0:SUB])
                else:
                    drain(eng, ob_c[:], ps[:, 0:SUB])

            # ---- stores: one full-width DMA per block; outC split across
            # both rings so the two final receipts overlap. outA and outB
            # both ride SWDGE (gpsimd is otherwise idle) so the scalar
            # ring's only tail job is its outC half — putting outB on
            # scalar delayed that half ~0.5us behind outB's issue.
            nc.gpsimd.dma_start(outA[:], ob_a[:])   # ready mid-stream; SWDGE
            nc.gpsimd.dma_start(outB[:], ob_b[:])   # after g5's copy
            nc.sync.dma_start(outC[0:64, :], ob_c[0:64, :])
            nc.scalar.dma_start(outC[64:128, :], ob_c[64:128, :])

    nc.compile()
    return nc


def _get_nc(mode: str):
    if mode not in _cache:
        _cache[mode] = _build(mode)
    return _cache[mode]


def _pack_core(xc):
    """[256, PPC] -> chunk-major [128, KC*PPC] (each chunk contiguous)."""
    parts = []
    for g in range(NG):
        blk = xc[:, g * GRP : (g + 1) * GRP].reshape(KC, 128, GRP)
        parts.append(blk.transpose(1, 0, 2).reshape(128, KC * GRP))
    return np.concatenate(parts, axis=1)


def run(x, capsules, trace=False, trace_cores=None, mode=None):
    """Shard, execute on 8 cores, gather. Returns (out, BassKernelResults)."""
    if mode is None:
        mode = MODE
    nc = _get_nc(mode)

    x = np.asarray(x, dtype=np.float32)
    capsules = np.asarray(capsules, dtype=np.float32)
    xf = np.ascontiguousarray(
        x.reshape(POS, F).astype(_NP_DT[mode], copy=False).T
    )  # [F, POS]
    # W_eff[f,d] = sum_c Wc[f,c,d]; pack rows f=(k,p) -> [128, KC*CAP_DIM]
    weff = capsules.reshape(F, NUM_CAPS, CAP_DIM).sum(axis=1)
    w_np = np.float16 if mode == "fp8" else _NP_DT[mode]
    win = np.ascontiguousarray(
        weff.reshape(KC, 128, CAP_DIM).transpose(1, 0, 2).reshape(128, KC * CAP_DIM)
    ).astype(w_np)

    in_maps = [
        {"xT": _pack_core(xf[:, c * PPC : (c + 1) * PPC]), "win": win}
        for c in range(N_CORES)
    ]
    res = run_bass_kernel_spmd(
        nc,
        in_maps,
        core_ids=list(range(N_CORES)),
        trace=trace,
        trace_cores=trace_cores,
    )
    # strip s of group g lives in dram rows 32s..32s+CAP_DIM
    rows = (32 * np.arange(4)[:, None] + np.arange(CAP_DIM)[None, :]).ravel()
    out = np.empty((POS, CAP_DIM), dtype=np.float32)
    for c in range(N_CORES):
        r = res.results[c]
        full = np.empty((4, CAP_DIM, NG, SUB), dtype=np.float32)  # s d g n
        full[:, :, 0:4] = (
            r["outA"][rows].reshape(4, CAP_DIM, 4, SUB).astype(np.float32)
        )
        full[:, :, 4:6] = (
            r["outB"][rows].reshape(4, CAP_DIM, 2, SUB).astype(np.float32)
        )
        full[:, :, 6] = r["outC"][rows].reshape(4, CAP_DIM, SUB).astype(np.float32)
        # position = g*1792 + s*448 + n
        out[c * PPC : (c + 1) * PPC] = (
            full.transpose(1, 2, 0, 3).reshape(CAP_DIM, PPC).T
        )
    return out.reshape(B, H, W, CAP_DIM), res


def kernel(x, capsules):
    out, _ = run(x, capsules)
    return out
